# revision 19
# baseline (speedup 1.0000x reference)
import sys
sys.path.insert(0, '/opt/trn_rl_repo')
"""Deformable-attention Bass kernel (one batch image per core), v2.

v2 changes vs v1: bf16 data path, natural-q overlapped with gathers,
vT-stationary AV with ones-block denominator replication, DRAM-broadcast
softmax normalization, pipelined attention units.
"""
import numpy as np
import ml_dtypes
import concourse.bass as bass
import concourse.tile as tile
from concourse import bacc, mybir

F32 = mybir.dt.float32
BF16 = mybir.dt.bfloat16
I32 = mybir.dt.int32
AF = mybir.ActivationFunctionType
OP = mybir.AluOpType

DIM = 512; INNER = 512; H = W = 64; S = H * W
G = 8; D = 64; HEADS = 8; GH = GW = 16; J = GH * GW
SCALE = D ** -0.5
C15 = 64.0 / 15.0
MROWS = S + 2  # per-group map rows incl front/back guard
BF = ml_dtypes.bfloat16


def host_constants():
    j_of = (np.arange(2)[None, :, None] * 128 + np.arange(128)[:, None, None])
    j_of = np.broadcast_to(j_of, (128, 2, 8)).reshape(128, 16)  # [p, m=t*8+g]
    meshA = (j_of // GW) * C15 - 0.5
    meshB = (j_of % GW) * C15 - 0.5
    return meshA.astype(np.float32), meshB.astype(np.float32)


def prep_weights(w_q, w_off1, b_off1, w_off2, w_kv, w_out, b_out):
    w_q = np.asarray(w_q, np.float32); w_kv = np.asarray(w_kv, np.float32)
    w_out = np.asarray(w_out, np.float32)
    W2 = np.zeros((2, INNER, G), np.float32)
    for g in range(G):
        for k in range(2):
            W2[k, g * D:(g + 1) * D, g] = np.asarray(w_off2, np.float32)[k]
    return {
        "w_qT": np.ascontiguousarray(w_q.T).astype(BF),
        "w_kT": np.ascontiguousarray(w_kv[:INNER].T).astype(BF),
        "w_vT": np.ascontiguousarray(w_kv[INNER:].T).astype(BF),
        "w_oT": np.ascontiguousarray(w_out.T).astype(BF),
        "w1v": np.ascontiguousarray(np.tile(np.asarray(w_off1, np.float32), G))[:, None],
        "b1v": np.ascontiguousarray(np.tile(np.asarray(b_off1, np.float32), G))[:, None],
        "W2x": np.ascontiguousarray(W2[0]),
        "W2y": np.ascontiguousarray(W2[1]),
        "b_out": np.asarray(b_out, np.float32)[:, None],
    }


def build(stage=5):
    nc = bacc.Bacc("TRN2", target_bir_lowering=False)
    x_in = nc.dram_tensor("x", [DIM, S], BF16, kind="ExternalInput")
    w_qT = nc.dram_tensor("w_qT", [DIM, INNER], BF16, kind="ExternalInput")
    w_kT = nc.dram_tensor("w_kT", [INNER, INNER], BF16, kind="ExternalInput")
    w_vT = nc.dram_tensor("w_vT", [INNER, INNER], BF16, kind="ExternalInput")
    w_oT = nc.dram_tensor("w_oT", [INNER, DIM], BF16, kind="ExternalInput")
    w1v = nc.dram_tensor("w1v", [INNER, 1], F32, kind="ExternalInput")
    b1v = nc.dram_tensor("b1v", [INNER, 1], F32, kind="ExternalInput")
    W2x = nc.dram_tensor("W2x", [INNER, G], F32, kind="ExternalInput")
    W2y = nc.dram_tensor("W2y", [INNER, G], F32, kind="ExternalInput")
    b_out = nc.dram_tensor("b_out", [DIM, 1], F32, kind="ExternalInput")
    y_out = nc.dram_tensor("y", [DIM, S], F32, kind="ExternalOutput")

    meshA_np, meshB_np = host_constants()
    meshA_d = nc.inline_tensor(meshA_np, "meshA")
    meshB_d = nc.inline_tensor(meshB_np, "meshB")

    with tile.TileContext(nc) as tc:
        # ---------------- persistent pool ----------------
        P0 = tc.alloc_tile_pool(name="P0", bufs=1)
        ident = P0.tile([128, 128], F32)
        from concourse.masks import make_identity
        make_identity(nc, ident[:])
        ident_bf = P0.tile([128, 128], BF16)
        nc.vector.tensor_copy(ident_bf[:], ident[:])
        meshA = P0.tile([128, 16], F32); meshB = P0.tile([128, 16], F32)
        nc.sync.dma_start(meshA[:], meshA_d.ap())
        nc.sync.dma_start(meshB[:], meshB_d.ap())
        w1_sb = P0.tile([128, 4], F32); b1_sb = P0.tile([128, 4], F32)
        nc.sync.dma_start(w1_sb[:], w1v.ap().rearrange("(c p) one -> p (c one)", p=128))
        nc.sync.dma_start(b1_sb[:], b1v.ap().rearrange("(c p) one -> p (c one)", p=128))
        W2x_sb = P0.tile([128, 4, G], F32); W2y_sb = P0.tile([128, 4, G], F32)
        nc.sync.dma_start(W2x_sb[:], W2x.ap().rearrange("(c p) g -> p c g", p=128))
        nc.sync.dma_start(W2y_sb[:], W2y.ap().rearrange("(c p) g -> p c g", p=128))
        bout_sb = P0.tile([128, 4], F32)
        nc.sync.dma_start(bout_sb[:], b_out.ap().rearrange("(c p) one -> p (c one)", p=128))
        IDX = P0.tile([128, 32], I32)
        Wb = P0.tile([128, 64], F32)
        kvf = P0.tile([128, 4, J], BF16)
        k_sb = P0.tile([128, 4, J], BF16)
        vT_sb = P0.tile([128, 2, 8, 128], BF16)   # [j, jt, head, (vT|ones)]
        wo_sb = P0.tile([128, 4, DIM], BF16)
        for c in range(4):
            nc.sync.dma_start(wo_sb[:, c, :], w_oT.ap()[c * 128:(c + 1) * 128, :])

        q_pool = tc.alloc_tile_pool(name="qp", bufs=1)
        q_sb = q_pool.tile([128, 4, S], BF16)

        # DRAM scratch
        drp = tc.alloc_tile_pool(name="dr", bufs=1, space="DRAM")
        qt_map = drp.tile([G * MROWS, D], F32)
        recd_p = tc.alloc_tile_pool(name="recd", bufs=3, space="DRAM")
        zt = P0.tile([G, 2, D], F32)
        nc.vector.memset(zt[:], 0.0)
        guard_dst = bass.AP(tensor=qt_map[:].tensor, offset=qt_map[:].offset,
                            ap=[[MROWS * D, G], [(MROWS - 1) * D, 2], [1, D]])
        nc.sync.dma_start(guard_dst, zt[:])

        # ---------------- phase A: load x, qT pass ----------------
        wq_pool = tc.alloc_tile_pool(name="wqp", bufs=1)
        wq_sb = wq_pool.tile([128, 4, INNER], BF16)
        for c in range(4):
            nc.sync.dma_start(wq_sb[:, c, :], w_qT.ap()[c * 128:(c + 1) * 128, :])
        x_pool = tc.alloc_tile_pool(name="xp", bufs=1)
        x_sb = x_pool.tile([128, 4, S], BF16)
        # s-block-major load so qT matmuls can start before the full x arrives
        for q4 in range(4):
            for c in range(4):
                nc.sync.dma_start(
                    x_sb[:, c, q4 * 1024:(q4 + 1) * 1024],
                    x_in.ap()[c * 128:(c + 1) * 128, q4 * 1024:(q4 + 1) * 1024])

        psA = tc.alloc_tile_pool(name="psA", bufs=3, space="PSUM")
        evp = tc.alloc_tile_pool(name="evp", bufs=4)
        for b in range(8):
            s0 = b * 512
            for ch in range(4):  # qT map chunks via matmul (lhsT = x chunk)
                cs = s0 + ch * 128
                pqt = psA.tile([128, INNER], F32, tag="pqt")
                for c in range(4):
                    nc.tensor.matmul(pqt[:], x_sb[:, c, cs:cs + 128], wq_sb[:, c, :],
                                     start=(c == 0), stop=(c == 3))
                ev = evp.tile([128, INNER], F32, tag="qt_ev")
                nc.scalar.activation(ev[:], pqt[:], AF.Copy)
                dst = bass.AP(tensor=qt_map[:].tensor,
                              offset=qt_map[:].offset + (1 + cs) * D,
                              ap=[[D, 128], [MROWS * D, G], [1, D]])
                nc.sync.dma_start(dst, ev[:].rearrange("p (g d) -> p g d", g=G))
        if stage < 2:
            psA.release(); evp.release(); x_pool.release(); wq_pool.release()
            recd_p.release(); drp.release(); q_pool.release(); P0.release()
            nc.compile(); return nc
        # ---------------- offsets ----------------
        offp = tc.alloc_tile_pool(name="offp", bufs=1)
        psOff = tc.alloc_tile_pool(name="psOff", bufs=1, space="PSUM")
        t_sb = offp.tile([128, 4, J], F32)
        for ic in range(4):
            pqd = psOff.tile([128, J], F32, tag="pqd")
            for c in range(4):
                base = x_sb[:, c, :]
                rhs = bass.AP(tensor=base.tensor, offset=base.offset,
                              ap=[list(base.ap[0]), [256, 16], [4, 16]])
                nc.tensor.matmul(pqd[:], wq_sb[:, c, ic * 128:(ic + 1) * 128], rhs,
                                 start=(c == 0), stop=(c == 3))
            nc.scalar.activation(t_sb[:, ic, :], pqd[:], AF.Gelu,
                                 bias=b1_sb[:, ic:ic + 1], scale=w1_sb[:, ic:ic + 1])
        offx = offp.tile([128, 16], F32); offy = offp.tile([128, 16], F32)
        for jt in range(2):
            pxt = psOff.tile([128, G], F32, tag="pxt")
            pyt = psOff.tile([128, G], F32, tag="pyt")
            px = pxt[:]; py = pyt[:]
            for c in range(4):
                nc.tensor.matmul(px, t_sb[:, c, jt * 128:(jt + 1) * 128],
                                 W2x_sb[:, c, :], start=(c == 0), stop=(c == 3))
            for c in range(4):
                nc.tensor.matmul(py, t_sb[:, c, jt * 128:(jt + 1) * 128],
                                 W2y_sb[:, c, :], start=(c == 0), stop=(c == 3))
            nc.scalar.activation(offx[:, jt * 8:(jt + 1) * 8], px, AF.Tanh)
            nc.scalar.activation(offy[:, jt * 8:(jt + 1) * 8], py, AF.Tanh)

        _fc = [0]
        def f16():
            _fc[0] += 1
            return offp.tile([128, 16], F32, name=f"f16_{_fc[0]}", tag=f"f16_{_fc[0]}")

        xs = f16(); ys = f16()
        nc.vector.scalar_tensor_tensor(out=xs[:], in0=offx[:], scalar=4.0 * C15,
                                       in1=meshA[:], op0=OP.mult, op1=OP.add)
        nc.vector.scalar_tensor_tensor(out=ys[:], in0=offy[:], scalar=4.0 * C15,
                                       in1=meshB[:], op0=OP.mult, op1=OP.add)

        def floor_of(src):
            _fc[0] += 1
            ti = offp.tile([128, 16], I32, name=f"i16_{_fc[0]}", tag=f"i16_{_fc[0]}")
            nc.vector.tensor_copy(ti[:], src)
            tf = f16()
            nc.vector.tensor_copy(tf[:], ti[:])
            gt = f16()
            nc.vector.tensor_tensor(out=gt[:], in0=tf[:], in1=src, op=OP.is_gt)
            fl = f16()
            nc.vector.tensor_tensor(out=fl[:], in0=tf[:], in1=gt[:], op=OP.subtract)
            return fl

        x0f = floor_of(xs[:]); y0f = floor_of(ys[:])

        def in_range(v, lo, hi):
            a = f16(); b2 = f16(); r = f16()
            nc.vector.tensor_scalar(out=a[:], in0=v, scalar1=float(lo), scalar2=None,
                                    op0=OP.is_ge)
            nc.vector.tensor_scalar(out=b2[:], in0=v, scalar1=float(hi), scalar2=None,
                                    op0=OP.is_le)
            nc.vector.tensor_tensor(out=r[:], in0=a[:], in1=b2[:], op=OP.mult)
            return r

        vx0 = in_range(x0f[:], 0, 63); vx1 = in_range(x0f[:], -1, 62)
        vy0 = in_range(y0f[:], 0, 63); vy1 = in_range(y0f[:], -1, 62)
        wx1 = f16(); wy1 = f16()
        nc.vector.tensor_tensor(out=wx1[:], in0=xs[:], in1=x0f[:], op=OP.subtract)
        nc.vector.tensor_tensor(out=wy1[:], in0=ys[:], in1=y0f[:], op=OP.subtract)
        wx0m = f16(); wx1m = f16(); wy0m = f16(); wy1m = f16()
        nc.vector.scalar_tensor_tensor(out=wx0m[:], in0=wx1[:], scalar=1.0,
                                       in1=vx0[:], op0=OP.subtract, op1=OP.mult)
        nc.vector.tensor_scalar_mul(wx0m[:], wx0m[:], -1.0)
        nc.vector.tensor_tensor(out=wx1m[:], in0=wx1[:], in1=vx1[:], op=OP.mult)
        nc.vector.scalar_tensor_tensor(out=wy0m[:], in0=wy1[:], scalar=1.0,
                                       in1=vy0[:], op0=OP.subtract, op1=OP.mult)
        nc.vector.tensor_scalar_mul(wy0m[:], wy0m[:], -1.0)
        nc.vector.tensor_tensor(out=wy1m[:], in0=wy1[:], in1=vy1[:], op=OP.mult)
        nc.vector.tensor_tensor(out=Wb[:, 0:16], in0=wy0m[:], in1=wx0m[:], op=OP.mult)
        nc.vector.tensor_tensor(out=Wb[:, 16:32], in0=wy0m[:], in1=wx1m[:], op=OP.mult)
        nc.vector.tensor_tensor(out=Wb[:, 32:48], in0=wy1m[:], in1=wx0m[:], op=OP.mult)
        nc.vector.tensor_tensor(out=Wb[:, 48:64], in0=wy1m[:], in1=wx1m[:], op=OP.mult)
        xm = f16(); ym0 = f16(); ym1 = f16()
        nc.vector.tensor_scalar(out=xm[:], in0=x0f[:], scalar1=-1.0, scalar2=63.0,
                                op0=OP.max, op1=OP.min)
        nc.vector.tensor_scalar(out=ym0[:], in0=y0f[:], scalar1=0.0, scalar2=63.0,
                                op0=OP.max, op1=OP.min)
        nc.vector.tensor_scalar(out=ym1[:], in0=y0f[:], scalar1=1.0, scalar2=0.0,
                                op0=OP.add, op1=OP.max)
        nc.vector.tensor_scalar_min(ym1[:], ym1[:], 63.0)
        IDXf = offp.tile([128, 32], F32)
        nc.vector.scalar_tensor_tensor(out=IDXf[:, 0:16], in0=ym0[:], scalar=64.0,
                                       in1=xm[:], op0=OP.mult, op1=OP.add)
        nc.vector.scalar_tensor_tensor(out=IDXf[:, 16:32], in0=ym1[:], scalar=64.0,
                                       in1=xm[:], op0=OP.mult, op1=OP.add)
        nc.vector.tensor_copy(IDX[:], IDXf[:])

        psOff.release(); psA.release()
        offp.release(); evp.release()

        if stage < 3:
            x_pool.release(); wq_pool.release()
            recd_p.release(); drp.release(); q_pool.release(); P0.release()
            nc.compile(); return nc

        # ---------------- natural q (overlaps the gathers below on PE) -----
        psQ = tc.alloc_tile_pool(name="psQ", bufs=3, space="PSUM")
        for b in range(8):
            s0 = b * 512
            for ic in range(4):
                pq = psQ.tile([128, 512], F32, tag="pq")
                for c in range(4):
                    nc.tensor.matmul(pq[:], wq_sb[:, c, ic * 128:(ic + 1) * 128],
                                     x_sb[:, c, s0:s0 + 512],
                                     start=(c == 0), stop=(c == 3))
                nc.scalar.activation(q_sb[:, ic, s0:s0 + 512], pq[:], AF.Copy)

        # ---------------- gathers + bilinear + kvf ----------------
        gpool = tc.alloc_tile_pool(name="gp", bufs=3)
        psT = tc.alloc_tile_pool(name="psT", bufs=2, space="PSUM")
        qt_flat = qt_map[:]
        for g in range(G):
            Gt = gpool.tile([128, 512], F32, tag="G")
            for yy in range(2):
                for t in range(2):
                    col = yy * 16 + t * 8 + g
                    nc.gpsimd.indirect_dma_start(
                        out=Gt[:, (yy * 2 + t) * 128:(yy * 2 + t + 1) * 128],
                        out_offset=None, in_=qt_flat,
                        in_offset=bass.IndirectOffsetOnAxis(
                            ap=IDX[:, col:col + 1], axis=0),
                        element_offset=(g * MROWS + 1) * D)
            for t in range(2):
                acc = gpool.tile([128, D], BF16, tag="acc")
                m = t * 8 + g
                nc.vector.tensor_scalar(out=acc[:], in0=Gt[:, t * 128:t * 128 + 64],
                                        scalar1=Wb[:, m:m + 1], scalar2=None,
                                        op0=OP.mult)
                for yy, xx in ((0, 1), (1, 0), (1, 1)):
                    blk = (yy * 2 + t) * 128 + xx * 64
                    wcol = (2 * yy + xx) * 16 + m
                    nc.vector.scalar_tensor_tensor(
                        out=acc[:], in0=Gt[:, blk:blk + 64],
                        scalar=Wb[:, wcol:wcol + 1], in1=acc[:],
                        op0=OP.mult, op1=OP.add)
                pt = psT.tile([64, 128], BF16, tag="pt")
                nc.tensor.transpose(pt[:], acc[:], ident_bf[:])
                nc.vector.tensor_copy(
                    kvf[(g % 2) * 64:(g % 2) * 64 + 64, g // 2, t * 128:(t + 1) * 128],
                    pt[:])

        if stage < 4:
            psT.release(); gpool.release(); psQ.release()
            x_pool.release(); wq_pool.release()
            recd_p.release(); drp.release(); q_pool.release(); P0.release()
            nc.compile(); return nc
        # ---------------- k and vT' ----------------
        psT.release(); gpool.release(); psQ.release()
        wkv_pool = tc.alloc_tile_pool(name="wkvp", bufs=1)
        wk_sb = wkv_pool.tile([128, 4, INNER], BF16)
        wv_sb = wkv_pool.tile([128, 4, INNER], BF16)
        for c in range(4):
            nc.sync.dma_start(wk_sb[:, c, :], w_kT.ap()[c * 128:(c + 1) * 128, :])
            nc.sync.dma_start(wv_sb[:, c, :], w_vT.ap()[c * 128:(c + 1) * 128, :])
        psKV = tc.alloc_tile_pool(name="psKV", bufs=2, space="PSUM")
        for oc in range(4):
            pk = psKV.tile([128, J], F32, tag="pk")
            for c in range(4):
                nc.tensor.matmul(pk[:], wk_sb[:, c, oc * 128:(oc + 1) * 128],
                                 kvf[:, c, :], start=(c == 0), stop=(c == 3))
            nc.vector.tensor_copy(k_sb[:, oc, :], pk[:])
        nc.vector.memset(vT_sb[:], 1.0)
        for jt in range(2):
            pv = psKV.tile([128, INNER], F32, tag="pv")
            for c in range(4):
                nc.tensor.matmul(pv[:], kvf[:, c, jt * 128:(jt + 1) * 128],
                                 wv_sb[:, c, :], start=(c == 0), stop=(c == 3))
            for h in range(8):
                # even head -> value cols 0:64 (den rows 64:128);
                # odd head  -> value cols 64:128 (den rows 0:64)
                o0 = (h % 2) * 64
                nc.vector.tensor_copy(vT_sb[:, jt, h, o0:o0 + 64],
                                      pv[:, h * 64:(h + 1) * 64])
        psKV.release(); wkv_pool.release()
        x_pool.release(); wq_pool.release()

        if stage < 5:
            recd_p.release(); drp.release(); q_pool.release(); P0.release()
            nc.compile(); return nc
        # ---------------- attention, pipelined units of (s-half, hp) -------
        outT_pool = tc.alloc_tile_pool(name="otp", bufs=1)
        outT_sb = outT_pool.tile([128, 4, S], BF16)
        ep = tc.alloc_tile_pool(name="ep", bufs=3)
        zrp = tc.alloc_tile_pool(name="zrp", bufs=4)
        pkp = tc.alloc_tile_pool(name="pkp", bufs=3)
        rcp = tc.alloc_tile_pool(name="rcp", bufs=3)
        yev = tc.alloc_tile_pool(name="yev", bufs=3)
        psF = tc.alloc_tile_pool(name="psF", bufs=1, space="PSUM")
        psS = tc.alloc_tile_pool(name="psS", bufs=1, space="PSUM")
        psAV = tc.alloc_tile_pool(name="psAV", bufs=3, space="PSUM")

        NU = 8  # units: u = sh*4 + hp (s-half major so final proj can overlap)
        st = {}  # per-unit tiles

        def issue_E_section(u, sec):
            sh, hp = u // 4, u % 4
            jt, blk = sec // 2, sec % 2
            if sec == 0:
                st[u] = {"E": ep.tile([128, 2, 2, 2048], BF16, tag="E",
                                      name=f"E{u}")}
            E = st[u]["E"]
            ps2 = psS.tile([128, 2, 1024], F32, tag="ps2", name=f"ps2_{u}_{sec}")
            for half in range(2):
                cs = sh * 2048 + blk * 1024 + half * 512
                nc.tensor.matmul(
                    ps2[:, 0, half * 512:(half + 1) * 512],
                    k_sb[0:64, hp, jt * 128:(jt + 1) * 128],
                    q_sb[0:64, hp, cs:cs + 512],
                    start=True, stop=True, tile_position=(0, 0))
                nc.tensor.matmul(
                    ps2[:, 1, half * 512:(half + 1) * 512],
                    k_sb[64:128, hp, jt * 128:(jt + 1) * 128],
                    q_sb[64:128, hp, cs:cs + 512],
                    start=True, stop=True, tile_position=(64, 0))
            eout = bass.AP(
                tensor=E[:].tensor, offset=E[:].offset + jt * 2048 + blk * 1024,
                ap=[list(E[:].ap[0]), [2 * 2048, 2], [1, 1024]])
            nc.scalar.activation(eout, ps2[:], AF.Exp, scale=SCALE)

        def issue_AV_part(u, sec):
            # 2 of the unit's 8 AV tiles per call: t = 2*sec, 2*sec+1
            sh, hp = u // 4, u % 4
            E = st[u]["E"]
            if sec == 0:
                st[u]["Z"] = zrp.tile([128, 8, 512], BF16, tag="Z", name=f"Z{u}")
            Z = st[u]["Z"]
            for t in (2 * sec, 2 * sec + 1):
                hh, chunk = t // 4, t % 4
                h = 2 * hp + hh
                pav = psAV.tile([128, 512], F32, tag="pav", name=f"pav{u}_{t}")
                for jt in range(2):
                    nc.tensor.matmul(pav[:], vT_sb[:, jt, h, :],
                                     E[:, hh, jt, chunk * 512:(chunk + 1) * 512],
                                     start=(jt == 0), stop=(jt == 1))
                nc.vector.tensor_copy(Z[:, t, :], pav[:])

        def issue_pack(u):
            # pack den rows (DMA only, off the DVE queue)
            Z = st[u]["Z"]
            pk = pkp.tile([8, 512], BF16, tag="pk8", name=f"pk8_{u}")
            # hh0 tiles (t=0..3): den replicated rows 64:128; hh1: rows 0:64
            nc.sync.dma_start(pk[0:4, :], Z[64:65, 0:4, :])
            nc.sync.dma_start(pk[4:8, :], Z[0:1, 4:8, :])
            st[u]["pk"] = pk

        def issue_recbc(u):
            # reciprocal + DRAM roundtrip broadcast
            rec = pkp.tile([8, 512], BF16, tag="rec8", name=f"rec8_{u}")
            with nc.allow_low_precision(reason="softmax denominators ~1e2, bf16 ok"):
                nc.vector.reciprocal(rec[:], st[u]["pk"][:])
            rd = recd_p.tile([8, 512], BF16, name=f"rd{u}")
            nc.sync.dma_start(rd[:], rec[:])
            REC = rcp.tile([128, 8, 512], BF16, tag="REC", name=f"REC{u}")
            src = bass.AP(tensor=rd[:].tensor, offset=rd[:].offset,
                          ap=[[0, 128], [512, 8], [1, 512]])
            nc.sync.dma_start(REC[:], src)
            st[u]["REC"] = REC

        def issue_mults(u):
            sh, hp = u // 4, u % 4
            Z = st[u]["Z"]; REC = st[u]["REC"]
            for t in range(8):
                hh, chunk = t // 4, t % 4
                r0 = hh * 64  # hh0 data rows 0:64, hh1 data rows 64:128
                cs = sh * 2048 + chunk * 512
                nc.vector.tensor_tensor(
                    out=outT_sb[r0:r0 + 64, hp, cs:cs + 512],
                    in0=Z[r0:r0 + 64, t, :], in1=REC[r0:r0 + 64, t, :],
                    op=OP.mult)

        def issue_final_group(sh, grp):
            # one (oc, sb2) group of the final projection for s-half sh
            oc, sbi = grp // 4, grp % 4
            sb2 = sh * 4 + sbi
            pf = psF.tile([128, 512], F32, tag="pf", name=f"pf{sh}_{grp}")
            for ic in range(4):
                nc.tensor.matmul(pf[:], wo_sb[:, ic, oc * 128:(oc + 1) * 128],
                                 outT_sb[:, ic, sb2 * 512:(sb2 + 1) * 512],
                                 start=(ic == 0), stop=(ic == 3))
            ye = yev.tile([128, 512], F32, tag="ye")
            nc.vector.tensor_scalar(out=ye[:], in0=pf[:],
                                    scalar1=bout_sb[:, oc:oc + 1], scalar2=None,
                                    op0=OP.add)
            nc.sync.dma_start(
                y_out.ap()[oc * 128:(oc + 1) * 128, sb2 * 512:(sb2 + 1) * 512],
                ye[:])

        # pipeline: E(u) | AV(u-1)+Zevac | pack(u-1) | recip/REC(u-2) | mults(u-3)
        # final proj for s-half 0 (ready after mults of unit 3, i.e. u>=7)
        # interleaves 2 groups per sec during u=7..8; s-half 1 runs at the end.
        for u in range(NU + 3):
            for sec in range(4):
                if u < NU:
                    issue_E_section(u, sec)
                if 0 <= u - 1 < NU:
                    issue_AV_part(u - 1, sec)
                if u in (7, 8):
                    g0 = (u - 7) * 8 + sec * 2
                    issue_final_group(0, g0)
                    issue_final_group(0, g0 + 1)
            if 0 <= u - 1 < NU:
                issue_pack(u - 1)
            if 0 <= u - 2 < NU:
                issue_recbc(u - 2)
            if 0 <= u - 3 < NU:
                issue_mults(u - 3)

        for grp in range(16):
            issue_final_group(1, grp)

        psAV.release(); psS.release(); psF.release()
        yev.release(); rcp.release(); pkp.release()
        zrp.release(); ep.release()
        outT_pool.release()
        recd_p.release(); drp.release(); q_pool.release(); P0.release()
    nc.compile()
    return nc


# ---------------------------------------------------------------------------
# Public entry point: full (unsharded) inputs -> full output.
# Data-parallel over batch: image i runs on NeuronCore i (8 cores).
# ---------------------------------------------------------------------------
_NC_CACHE = {}


def _get_nc():
    if "nc" not in _NC_CACHE:
        _NC_CACHE["nc"] = build()
    return _NC_CACHE["nc"]


def kernel(x, w_q, w_off1, b_off1, w_off2, w_kv, w_out, b_out):
    from concourse.bass_utils import run_bass_kernel_spmd
    x = np.asarray(x, np.float32)
    b = x.shape[0]
    assert x.shape == (8, DIM, H, W), f"unexpected x shape {x.shape}"
    wd = prep_weights(w_q, w_off1, b_off1, w_off2, w_kv, w_out, b_out)
    in_maps = [{"x": np.ascontiguousarray(x[i].reshape(DIM, S)).astype(BF), **wd}
               for i in range(b)]
    nc = _get_nc()
    res = run_bass_kernel_spmd(nc, in_maps, core_ids=list(range(b)))
    out = np.stack([res.results[i]["y"].reshape(DIM, H, W) for i in range(b)])
    return out.astype(np.float32)


# revision 20
# speedup vs baseline: 1.1025x; 1.1025x over previous
import sys
sys.path.insert(0, '/opt/trn_rl_repo')
"""Deformable-attention Bass kernel (one batch image per core), v2.

v2 changes vs v1: bf16 data path, natural-q overlapped with gathers,
vT-stationary AV with ones-block denominator replication, DRAM-broadcast
softmax normalization, pipelined attention units.
"""
import numpy as np
import ml_dtypes
import concourse.bass as bass
import concourse.tile as tile
from concourse import bacc, mybir

F32 = mybir.dt.float32
BF16 = mybir.dt.bfloat16
I32 = mybir.dt.int32
AF = mybir.ActivationFunctionType
OP = mybir.AluOpType

DIM = 512; INNER = 512; H = W = 64; S = H * W
G = 8; D = 64; HEADS = 8; GH = GW = 16; J = GH * GW
SCALE = D ** -0.5
C15 = 64.0 / 15.0
MROWS = S + 2  # per-group map rows incl front/back guard
BF = ml_dtypes.bfloat16


def host_constants():
    j_of = (np.arange(2)[None, :, None] * 128 + np.arange(128)[:, None, None])
    j_of = np.broadcast_to(j_of, (128, 2, 8)).reshape(128, 16)  # [p, m=t*8+g]
    meshA = (j_of // GW) * C15 - 0.5
    meshB = (j_of % GW) * C15 - 0.5
    return meshA.astype(np.float32), meshB.astype(np.float32)


def prep_weights(w_q, w_off1, b_off1, w_off2, w_kv, w_out, b_out):
    w_q = np.asarray(w_q, np.float32); w_kv = np.asarray(w_kv, np.float32)
    w_out = np.asarray(w_out, np.float32)
    W2 = np.zeros((2, INNER, G), np.float32)
    for g in range(G):
        for k in range(2):
            W2[k, g * D:(g + 1) * D, g] = np.asarray(w_off2, np.float32)[k]
    return {
        "w_qT": np.ascontiguousarray(w_q.T).astype(BF),
        "w_kT": np.ascontiguousarray(w_kv[:INNER].T).astype(BF),
        "w_vT": np.ascontiguousarray(w_kv[INNER:].T).astype(BF),
        "w_oT": np.ascontiguousarray(w_out.T).astype(BF),
        "w1v": np.ascontiguousarray(np.tile(np.asarray(w_off1, np.float32), G))[:, None],
        "b1v": np.ascontiguousarray(np.tile(np.asarray(b_off1, np.float32), G))[:, None],
        "W2x": np.ascontiguousarray(W2[0]),
        "W2y": np.ascontiguousarray(W2[1]),
        "b_out": np.asarray(b_out, np.float32)[:, None],
    }


def build(stage=5):
    nc = bacc.Bacc("TRN2", target_bir_lowering=False)
    x_in = nc.dram_tensor("x", [DIM, S], BF16, kind="ExternalInput")
    w_qT = nc.dram_tensor("w_qT", [DIM, INNER], BF16, kind="ExternalInput")
    w_kT = nc.dram_tensor("w_kT", [INNER, INNER], BF16, kind="ExternalInput")
    w_vT = nc.dram_tensor("w_vT", [INNER, INNER], BF16, kind="ExternalInput")
    w_oT = nc.dram_tensor("w_oT", [INNER, DIM], BF16, kind="ExternalInput")
    w1v = nc.dram_tensor("w1v", [INNER, 1], F32, kind="ExternalInput")
    b1v = nc.dram_tensor("b1v", [INNER, 1], F32, kind="ExternalInput")
    W2x = nc.dram_tensor("W2x", [INNER, G], F32, kind="ExternalInput")
    W2y = nc.dram_tensor("W2y", [INNER, G], F32, kind="ExternalInput")
    b_out = nc.dram_tensor("b_out", [DIM, 1], F32, kind="ExternalInput")
    y_out = nc.dram_tensor("y", [DIM, S], F32, kind="ExternalOutput")

    meshA_np, meshB_np = host_constants()
    meshA_d = nc.inline_tensor(meshA_np, "meshA")
    meshB_d = nc.inline_tensor(meshB_np, "meshB")

    with tile.TileContext(nc) as tc:
        # ---------------- persistent pool ----------------
        P0 = tc.alloc_tile_pool(name="P0", bufs=1)
        ident = P0.tile([128, 128], F32)
        from concourse.masks import make_identity
        make_identity(nc, ident[:])
        ident_bf = P0.tile([128, 128], BF16)
        nc.vector.tensor_copy(ident_bf[:], ident[:])
        meshA = P0.tile([128, 16], F32); meshB = P0.tile([128, 16], F32)
        nc.sync.dma_start(meshA[:], meshA_d.ap())
        nc.sync.dma_start(meshB[:], meshB_d.ap())
        w1_sb = P0.tile([128, 4], F32); b1_sb = P0.tile([128, 4], F32)
        nc.sync.dma_start(w1_sb[:], w1v.ap().rearrange("(c p) one -> p (c one)", p=128))
        nc.sync.dma_start(b1_sb[:], b1v.ap().rearrange("(c p) one -> p (c one)", p=128))
        W2x_sb = P0.tile([128, 4, G], F32); W2y_sb = P0.tile([128, 4, G], F32)
        nc.sync.dma_start(W2x_sb[:], W2x.ap().rearrange("(c p) g -> p c g", p=128))
        nc.sync.dma_start(W2y_sb[:], W2y.ap().rearrange("(c p) g -> p c g", p=128))
        bout_sb = P0.tile([128, 4], F32)
        nc.sync.dma_start(bout_sb[:], b_out.ap().rearrange("(c p) one -> p (c one)", p=128))
        IDX = P0.tile([128, 32], I32)
        Wb = P0.tile([128, 64], F32)
        kvf = P0.tile([128, 4, J], BF16)
        k_sb = P0.tile([128, 4, J], BF16)
        vT_sb = P0.tile([128, 2, 8, 128], BF16)   # [j, jt, head, (vT|ones)]
        wo_sb = P0.tile([128, 4, DIM], BF16)
        for c in range(4):
            nc.sync.dma_start(wo_sb[:, c, :], w_oT.ap()[c * 128:(c + 1) * 128, :])

        q_pool = tc.alloc_tile_pool(name="qp", bufs=1)
        q_sb = q_pool.tile([128, 4, S], BF16)

        # DRAM scratch
        drp = tc.alloc_tile_pool(name="dr", bufs=1, space="DRAM")
        qt_map = drp.tile([G * MROWS, D], F32)
        recd_p = tc.alloc_tile_pool(name="recd", bufs=3, space="DRAM")
        zt = P0.tile([G, 2, D], F32)
        nc.vector.memset(zt[:], 0.0)
        guard_dst = bass.AP(tensor=qt_map[:].tensor, offset=qt_map[:].offset,
                            ap=[[MROWS * D, G], [(MROWS - 1) * D, 2], [1, D]])
        nc.sync.dma_start(guard_dst, zt[:])

        # ---------------- phase A: load x, qT pass ----------------
        wq_pool = tc.alloc_tile_pool(name="wqp", bufs=1)
        wq_sb = wq_pool.tile([128, 4, INNER], BF16)
        for c in range(4):
            nc.sync.dma_start(wq_sb[:, c, :], w_qT.ap()[c * 128:(c + 1) * 128, :])
        x_pool = tc.alloc_tile_pool(name="xp", bufs=1)
        x_sb = x_pool.tile([128, 4, S], BF16)
        # s-block-major load so qT matmuls can start before the full x arrives
        for q4 in range(4):
            for c in range(4):
                nc.sync.dma_start(
                    x_sb[:, c, q4 * 1024:(q4 + 1) * 1024],
                    x_in.ap()[c * 128:(c + 1) * 128, q4 * 1024:(q4 + 1) * 1024])

        psA = tc.alloc_tile_pool(name="psA", bufs=3, space="PSUM")
        evp = tc.alloc_tile_pool(name="evp", bufs=4)
        for b in range(8):
            s0 = b * 512
            for ch in range(4):  # qT map chunks via matmul (lhsT = x chunk)
                cs = s0 + ch * 128
                pqt = psA.tile([128, INNER], F32, tag="pqt")
                for c in range(4):
                    nc.tensor.matmul(pqt[:], x_sb[:, c, cs:cs + 128], wq_sb[:, c, :],
                                     start=(c == 0), stop=(c == 3))
                ev = evp.tile([128, INNER], F32, tag="qt_ev")
                nc.scalar.activation(ev[:], pqt[:], AF.Copy)
                dst = bass.AP(tensor=qt_map[:].tensor,
                              offset=qt_map[:].offset + (1 + cs) * D,
                              ap=[[D, 128], [MROWS * D, G], [1, D]])
                nc.sync.dma_start(dst, ev[:].rearrange("p (g d) -> p g d", g=G))
        if stage < 2:
            psA.release(); evp.release(); x_pool.release(); wq_pool.release()
            recd_p.release(); drp.release(); q_pool.release(); P0.release()
            nc.compile(); return nc
        # ---------------- offsets ----------------
        offp = tc.alloc_tile_pool(name="offp", bufs=1)
        psOff = tc.alloc_tile_pool(name="psOff", bufs=1, space="PSUM")
        t_sb = offp.tile([128, 4, J], F32)
        for ic in range(4):
            pqd = psOff.tile([128, J], F32, tag="pqd")
            for c in range(4):
                base = x_sb[:, c, :]
                rhs = bass.AP(tensor=base.tensor, offset=base.offset,
                              ap=[list(base.ap[0]), [256, 16], [4, 16]])
                nc.tensor.matmul(pqd[:], wq_sb[:, c, ic * 128:(ic + 1) * 128], rhs,
                                 start=(c == 0), stop=(c == 3))
            nc.scalar.activation(t_sb[:, ic, :], pqd[:], AF.Gelu,
                                 bias=b1_sb[:, ic:ic + 1], scale=w1_sb[:, ic:ic + 1])
        offx = offp.tile([128, 16], F32); offy = offp.tile([128, 16], F32)
        for jt in range(2):
            pxt = psOff.tile([128, G], F32, tag="pxt")
            pyt = psOff.tile([128, G], F32, tag="pyt")
            px = pxt[:]; py = pyt[:]
            for c in range(4):
                nc.tensor.matmul(px, t_sb[:, c, jt * 128:(jt + 1) * 128],
                                 W2x_sb[:, c, :], start=(c == 0), stop=(c == 3))
            for c in range(4):
                nc.tensor.matmul(py, t_sb[:, c, jt * 128:(jt + 1) * 128],
                                 W2y_sb[:, c, :], start=(c == 0), stop=(c == 3))
            nc.scalar.activation(offx[:, jt * 8:(jt + 1) * 8], px, AF.Tanh)
            nc.scalar.activation(offy[:, jt * 8:(jt + 1) * 8], py, AF.Tanh)

        _fc = [0]
        def f16():
            _fc[0] += 1
            return offp.tile([128, 16], F32, name=f"f16_{_fc[0]}", tag=f"f16_{_fc[0]}")

        xs = f16(); ys = f16()
        nc.vector.scalar_tensor_tensor(out=xs[:], in0=offx[:], scalar=4.0 * C15,
                                       in1=meshA[:], op0=OP.mult, op1=OP.add)
        nc.vector.scalar_tensor_tensor(out=ys[:], in0=offy[:], scalar=4.0 * C15,
                                       in1=meshB[:], op0=OP.mult, op1=OP.add)

        def floor_of(src):
            _fc[0] += 1
            ti = offp.tile([128, 16], I32, name=f"i16_{_fc[0]}", tag=f"i16_{_fc[0]}")
            nc.vector.tensor_copy(ti[:], src)
            tf = f16()
            nc.vector.tensor_copy(tf[:], ti[:])
            gt = f16()
            nc.vector.tensor_tensor(out=gt[:], in0=tf[:], in1=src, op=OP.is_gt)
            fl = f16()
            nc.vector.tensor_tensor(out=fl[:], in0=tf[:], in1=gt[:], op=OP.subtract)
            return fl

        x0f = floor_of(xs[:]); y0f = floor_of(ys[:])

        def in_range(v, lo, hi):
            a = f16(); b2 = f16(); r = f16()
            nc.vector.tensor_scalar(out=a[:], in0=v, scalar1=float(lo), scalar2=None,
                                    op0=OP.is_ge)
            nc.vector.tensor_scalar(out=b2[:], in0=v, scalar1=float(hi), scalar2=None,
                                    op0=OP.is_le)
            nc.vector.tensor_tensor(out=r[:], in0=a[:], in1=b2[:], op=OP.mult)
            return r

        vx0 = in_range(x0f[:], 0, 63); vx1 = in_range(x0f[:], -1, 62)
        vy0 = in_range(y0f[:], 0, 63); vy1 = in_range(y0f[:], -1, 62)
        wx1 = f16(); wy1 = f16()
        nc.vector.tensor_tensor(out=wx1[:], in0=xs[:], in1=x0f[:], op=OP.subtract)
        nc.vector.tensor_tensor(out=wy1[:], in0=ys[:], in1=y0f[:], op=OP.subtract)
        wx0m = f16(); wx1m = f16(); wy0m = f16(); wy1m = f16()
        nc.vector.scalar_tensor_tensor(out=wx0m[:], in0=wx1[:], scalar=1.0,
                                       in1=vx0[:], op0=OP.subtract, op1=OP.mult)
        nc.vector.tensor_scalar_mul(wx0m[:], wx0m[:], -1.0)
        nc.vector.tensor_tensor(out=wx1m[:], in0=wx1[:], in1=vx1[:], op=OP.mult)
        nc.vector.scalar_tensor_tensor(out=wy0m[:], in0=wy1[:], scalar=1.0,
                                       in1=vy0[:], op0=OP.subtract, op1=OP.mult)
        nc.vector.tensor_scalar_mul(wy0m[:], wy0m[:], -1.0)
        nc.vector.tensor_tensor(out=wy1m[:], in0=wy1[:], in1=vy1[:], op=OP.mult)
        nc.vector.tensor_tensor(out=Wb[:, 0:16], in0=wy0m[:], in1=wx0m[:], op=OP.mult)
        nc.vector.tensor_tensor(out=Wb[:, 16:32], in0=wy0m[:], in1=wx1m[:], op=OP.mult)
        nc.vector.tensor_tensor(out=Wb[:, 32:48], in0=wy1m[:], in1=wx0m[:], op=OP.mult)
        nc.vector.tensor_tensor(out=Wb[:, 48:64], in0=wy1m[:], in1=wx1m[:], op=OP.mult)
        xm = f16(); ym0 = f16(); ym1 = f16()
        nc.vector.tensor_scalar(out=xm[:], in0=x0f[:], scalar1=-1.0, scalar2=63.0,
                                op0=OP.max, op1=OP.min)
        nc.vector.tensor_scalar(out=ym0[:], in0=y0f[:], scalar1=0.0, scalar2=63.0,
                                op0=OP.max, op1=OP.min)
        nc.vector.tensor_scalar(out=ym1[:], in0=y0f[:], scalar1=1.0, scalar2=0.0,
                                op0=OP.add, op1=OP.max)
        nc.vector.tensor_scalar_min(ym1[:], ym1[:], 63.0)
        IDXf = offp.tile([128, 32], F32)
        nc.vector.scalar_tensor_tensor(out=IDXf[:, 0:16], in0=ym0[:], scalar=64.0,
                                       in1=xm[:], op0=OP.mult, op1=OP.add)
        nc.vector.scalar_tensor_tensor(out=IDXf[:, 16:32], in0=ym1[:], scalar=64.0,
                                       in1=xm[:], op0=OP.mult, op1=OP.add)
        nc.vector.tensor_copy(IDX[:], IDXf[:])

        psOff.release(); psA.release()
        offp.release(); evp.release()

        if stage < 3:
            x_pool.release(); wq_pool.release()
            recd_p.release(); drp.release(); q_pool.release(); P0.release()
            nc.compile(); return nc

        # ---------------- natural q (overlaps the gathers below on PE) -----
        psQ = tc.alloc_tile_pool(name="psQ", bufs=3, space="PSUM")
        for b in range(8):
            s0 = b * 512
            for ic in range(4):
                pq = psQ.tile([128, 512], F32, tag="pq")
                for c in range(4):
                    nc.tensor.matmul(pq[:], wq_sb[:, c, ic * 128:(ic + 1) * 128],
                                     x_sb[:, c, s0:s0 + 512],
                                     start=(c == 0), stop=(c == 3))
                nc.scalar.activation(q_sb[:, ic, s0:s0 + 512], pq[:], AF.Copy)

        # ---------------- gathers + bilinear + kvf ----------------
        gpool = tc.alloc_tile_pool(name="gp", bufs=3)
        psT = tc.alloc_tile_pool(name="psT", bufs=2, space="PSUM")
        qt_flat = qt_map[:]
        for g in range(G):
            Gt = gpool.tile([128, 512], F32, tag="G")
            for yy in range(2):
                for t in range(2):
                    col = yy * 16 + t * 8 + g
                    nc.gpsimd.indirect_dma_start(
                        out=Gt[:, (yy * 2 + t) * 128:(yy * 2 + t + 1) * 128],
                        out_offset=None, in_=qt_flat,
                        in_offset=bass.IndirectOffsetOnAxis(
                            ap=IDX[:, col:col + 1], axis=0),
                        element_offset=(g * MROWS + 1) * D)
            for t in range(2):
                acc = gpool.tile([128, D], BF16, tag="acc")
                m = t * 8 + g
                nc.vector.tensor_scalar(out=acc[:], in0=Gt[:, t * 128:t * 128 + 64],
                                        scalar1=Wb[:, m:m + 1], scalar2=None,
                                        op0=OP.mult)
                for yy, xx in ((0, 1), (1, 0), (1, 1)):
                    blk = (yy * 2 + t) * 128 + xx * 64
                    wcol = (2 * yy + xx) * 16 + m
                    nc.vector.scalar_tensor_tensor(
                        out=acc[:], in0=Gt[:, blk:blk + 64],
                        scalar=Wb[:, wcol:wcol + 1], in1=acc[:],
                        op0=OP.mult, op1=OP.add)
                pt = psT.tile([64, 128], BF16, tag="pt")
                nc.tensor.transpose(pt[:], acc[:], ident_bf[:])
                nc.vector.tensor_copy(
                    kvf[(g % 2) * 64:(g % 2) * 64 + 64, g // 2, t * 128:(t + 1) * 128],
                    pt[:])

        if stage < 4:
            psT.release(); gpool.release(); psQ.release()
            x_pool.release(); wq_pool.release()
            recd_p.release(); drp.release(); q_pool.release(); P0.release()
            nc.compile(); return nc
        # ---------------- k and vT' ----------------
        psT.release(); gpool.release(); psQ.release()
        wkv_pool = tc.alloc_tile_pool(name="wkvp", bufs=1)
        wk_sb = wkv_pool.tile([128, 4, INNER], BF16)
        wv_sb = wkv_pool.tile([128, 4, INNER], BF16)
        for c in range(4):
            nc.sync.dma_start(wk_sb[:, c, :], w_kT.ap()[c * 128:(c + 1) * 128, :])
            nc.sync.dma_start(wv_sb[:, c, :], w_vT.ap()[c * 128:(c + 1) * 128, :])
        psKV = tc.alloc_tile_pool(name="psKV", bufs=2, space="PSUM")
        for oc in range(4):
            pk = psKV.tile([128, J], F32, tag="pk")
            for c in range(4):
                nc.tensor.matmul(pk[:], wk_sb[:, c, oc * 128:(oc + 1) * 128],
                                 kvf[:, c, :], start=(c == 0), stop=(c == 3))
            nc.vector.tensor_copy(k_sb[:, oc, :], pk[:])
        nc.vector.memset(vT_sb[:], 1.0)
        for jt in range(2):
            pv = psKV.tile([128, INNER], F32, tag="pv")
            for c in range(4):
                nc.tensor.matmul(pv[:], kvf[:, c, jt * 128:(jt + 1) * 128],
                                 wv_sb[:, c, :], start=(c == 0), stop=(c == 3))
            for h in range(8):
                # even head -> value cols 0:64 (den rows 64:128);
                # odd head  -> value cols 64:128 (den rows 0:64)
                o0 = (h % 2) * 64
                nc.vector.tensor_copy(vT_sb[:, jt, h, o0:o0 + 64],
                                      pv[:, h * 64:(h + 1) * 64])
        psKV.release(); wkv_pool.release()
        x_pool.release(); wq_pool.release()

        if stage < 5:
            recd_p.release(); drp.release(); q_pool.release(); P0.release()
            nc.compile(); return nc
        # ---------------- attention, pipelined units of (s-half, hp) -------
        outT_pool = tc.alloc_tile_pool(name="otp", bufs=1)
        outT_sb = outT_pool.tile([128, 4, S], BF16)
        ep = tc.alloc_tile_pool(name="ep", bufs=3)
        zrp = tc.alloc_tile_pool(name="zrp", bufs=4)
        pkp = tc.alloc_tile_pool(name="pkp", bufs=3)
        rcp = tc.alloc_tile_pool(name="rcp", bufs=3)
        yev = tc.alloc_tile_pool(name="yev", bufs=3)
        psF = tc.alloc_tile_pool(name="psF", bufs=1, space="PSUM")
        psS = tc.alloc_tile_pool(name="psS", bufs=2, space="PSUM")
        psAV = tc.alloc_tile_pool(name="psAV", bufs=3, space="PSUM")

        NU = 8  # units: u = sh*4 + hp (s-half major so final proj can overlap)
        st = {}  # per-unit tiles

        def issue_E_section(u, sec):
            sh, hp = u // 4, u % 4
            jt, blk = sec // 4, sec % 4
            if sec == 0:
                st[u] = {"E": ep.tile([128, 2, 2, 2048], BF16, tag="E",
                                      name=f"E{u}")}
            E = st[u]["E"]
            ps2 = psS.tile([128, 2, 512], F32, tag="ps2", name=f"ps2_{u}_{sec}")
            cs = sh * 2048 + blk * 512
            nc.tensor.matmul(
                ps2[:, 0, :],
                k_sb[0:64, hp, jt * 128:(jt + 1) * 128],
                q_sb[0:64, hp, cs:cs + 512],
                start=True, stop=True, tile_position=(0, 0))
            nc.tensor.matmul(
                ps2[:, 1, :],
                k_sb[64:128, hp, jt * 128:(jt + 1) * 128],
                q_sb[64:128, hp, cs:cs + 512],
                start=True, stop=True, tile_position=(64, 0))
            eout = bass.AP(
                tensor=E[:].tensor, offset=E[:].offset + jt * 2048 + blk * 512,
                ap=[list(E[:].ap[0]), [2 * 2048, 2], [1, 512]])
            nc.scalar.activation(eout, ps2[:], AF.Exp, scale=SCALE)

        def issue_AV_part(u, sec):
            # one of the unit's 8 AV tiles per call: t = sec
            sh, hp = u // 4, u % 4
            E = st[u]["E"]
            if sec == 0:
                st[u]["Z"] = zrp.tile([128, 8, 512], BF16, tag="Z", name=f"Z{u}")
            Z = st[u]["Z"]
            t = sec
            hh, chunk = t // 4, t % 4
            h = 2 * hp + hh
            pav = psAV.tile([128, 512], F32, tag="pav", name=f"pav{u}_{t}")
            for jt in range(2):
                nc.tensor.matmul(pav[:], vT_sb[:, jt, h, :],
                                 E[:, hh, jt, chunk * 512:(chunk + 1) * 512],
                                 start=(jt == 0), stop=(jt == 1))
            nc.vector.tensor_copy(Z[:, t, :], pav[:])

        def issue_pack(u):
            # pack den rows (gpsimd cast DMA bf16->f32, off the DVE queue)
            Z = st[u]["Z"]
            pk = pkp.tile([8, 512], F32, tag="pk8", name=f"pk8_{u}")
            # hh0 tiles (t=0..3): den replicated rows 64:128; hh1: rows 0:64
            nc.gpsimd.dma_start(pk[0:4, :], Z[64:65, 0:4, :])
            nc.gpsimd.dma_start(pk[4:8, :], Z[0:1, 4:8, :])
            st[u]["pk"] = pk

        def issue_recbc(u):
            # fast approx reciprocal (f32) + DRAM roundtrip broadcast (bf16)
            rec = pkp.tile([8, 512], F32, tag="rec8", name=f"rec8_{u}")
            nc.vector.reciprocal_approx_fast(rec[:], st[u]["pk"][:])
            rd = recd_p.tile([8, 512], BF16, name=f"rd{u}")
            nc.gpsimd.dma_start(rd[:], rec[:])
            REC = rcp.tile([128, 8, 512], BF16, tag="REC", name=f"REC{u}")
            src = bass.AP(tensor=rd[:].tensor, offset=rd[:].offset,
                          ap=[[0, 128], [512, 8], [1, 512]])
            nc.sync.dma_start(REC[:], src)
            st[u]["REC"] = REC

        def issue_mults(u):
            sh, hp = u // 4, u % 4
            Z = st[u]["Z"]; REC = st[u]["REC"]
            for hh in range(2):
                r0 = hh * 64  # hh0 data rows 0:64, hh1 data rows 64:128
                t0 = hh * 4
                cs = sh * 2048
                nc.vector.tensor_tensor(
                    out=outT_sb[r0:r0 + 64, hp, cs:cs + 2048],
                    in0=Z[r0:r0 + 64, t0:t0 + 4, :], in1=REC[r0:r0 + 64, t0:t0 + 4, :],
                    op=OP.mult)

        def issue_final_group(sh, grp):
            # one (oc, sb2) group of the final projection for s-half sh
            oc, sbi = grp // 4, grp % 4
            sb2 = sh * 4 + sbi
            pf = psF.tile([128, 512], F32, tag="pf", name=f"pf{sh}_{grp}")
            for ic in range(4):
                nc.tensor.matmul(pf[:], wo_sb[:, ic, oc * 128:(oc + 1) * 128],
                                 outT_sb[:, ic, sb2 * 512:(sb2 + 1) * 512],
                                 start=(ic == 0), stop=(ic == 3))
            ye = yev.tile([128, 512], F32, tag="ye")
            nc.vector.tensor_scalar(out=ye[:], in0=pf[:],
                                    scalar1=bout_sb[:, oc:oc + 1], scalar2=None,
                                    op0=OP.add)
            nc.scalar.dma_start(
                y_out.ap()[oc * 128:(oc + 1) * 128, sb2 * 512:(sb2 + 1) * 512],
                ye[:])

        # pipeline: E(u) | AV(u-1)+Zevac | pack(u-1) | recip/REC(u-2) | mults(u-3)
        # final proj for s-half 0 (ready after mults of unit 3, i.e. u>=7)
        # interleaves 2 groups per sec during u=7..8; s-half 1 runs at the end.
        for u in range(NU + 3):
            for sec in range(8):
                if u < NU:
                    issue_E_section(u, sec)
                if 0 <= u - 1 < NU:
                    issue_AV_part(u - 1, sec)
                if u in (7, 8):
                    issue_final_group(0, (u - 7) * 8 + sec)
            if 0 <= u - 1 < NU:
                issue_pack(u - 1)
            if 0 <= u - 2 < NU:
                issue_recbc(u - 2)
            if 0 <= u - 3 < NU:
                issue_mults(u - 3)

        for grp in range(16):
            issue_final_group(1, grp)

        psAV.release(); psS.release(); psF.release()
        yev.release(); rcp.release(); pkp.release()
        zrp.release(); ep.release()
        outT_pool.release()
        recd_p.release(); drp.release(); q_pool.release(); P0.release()
    nc.compile()
    return nc


# ---------------------------------------------------------------------------
# Public entry point: full (unsharded) inputs -> full output.
# Data-parallel over batch: image i runs on NeuronCore i (8 cores).
# ---------------------------------------------------------------------------
_NC_CACHE = {}


def _get_nc():
    if "nc" not in _NC_CACHE:
        _NC_CACHE["nc"] = build()
    return _NC_CACHE["nc"]


def kernel(x, w_q, w_off1, b_off1, w_off2, w_kv, w_out, b_out):
    from concourse.bass_utils import run_bass_kernel_spmd
    x = np.asarray(x, np.float32)
    b = x.shape[0]
    assert x.shape == (8, DIM, H, W), f"unexpected x shape {x.shape}"
    wd = prep_weights(w_q, w_off1, b_off1, w_off2, w_kv, w_out, b_out)
    in_maps = [{"x": np.ascontiguousarray(x[i].reshape(DIM, S)).astype(BF), **wd}
               for i in range(b)]
    nc = _get_nc()
    res = run_bass_kernel_spmd(nc, in_maps, core_ids=list(range(b)))
    out = np.stack([res.results[i]["y"].reshape(DIM, H, W) for i in range(b)])
    return out.astype(np.float32)


# revision 23
# speedup vs baseline: 1.1263x; 1.0216x over previous
import sys
sys.path.insert(0, '/opt/trn_rl_repo')
"""Deformable-attention Bass kernel (one batch image per core), v2.

v2 changes vs v1: bf16 data path, natural-q overlapped with gathers,
vT-stationary AV with ones-block denominator replication, DRAM-broadcast
softmax normalization, pipelined attention units.
"""
import numpy as np
import ml_dtypes
import concourse.bass as bass
import concourse.tile as tile
from concourse import bacc, mybir

F32 = mybir.dt.float32
BF16 = mybir.dt.bfloat16
I32 = mybir.dt.int32
AF = mybir.ActivationFunctionType
OP = mybir.AluOpType

DIM = 512; INNER = 512; H = W = 64; S = H * W
G = 8; D = 64; HEADS = 8; GH = GW = 16; J = GH * GW
SCALE = D ** -0.5
C15 = 64.0 / 15.0
MROWS = S + 2  # per-group map rows incl front/back guard
BF = ml_dtypes.bfloat16


def host_constants():
    j_of = (np.arange(2)[None, :, None] * 128 + np.arange(128)[:, None, None])
    j_of = np.broadcast_to(j_of, (128, 2, 8)).reshape(128, 16)  # [p, m=t*8+g]
    meshA = (j_of // GW) * C15 - 0.5
    meshB = (j_of % GW) * C15 - 0.5
    return meshA.astype(np.float32), meshB.astype(np.float32)


def prep_weights(w_q, w_off1, b_off1, w_off2, w_kv, w_out, b_out):
    w_q = np.asarray(w_q, np.float32); w_kv = np.asarray(w_kv, np.float32)
    w_out = np.asarray(w_out, np.float32)
    W2 = np.zeros((2, INNER, G), np.float32)
    for g in range(G):
        for k in range(2):
            W2[k, g * D:(g + 1) * D, g] = np.asarray(w_off2, np.float32)[k]
    return {
        "w_qT": np.ascontiguousarray(w_q.T).astype(BF),
        "w_kT": np.ascontiguousarray(w_kv[:INNER].T).astype(BF),
        "w_vT": np.ascontiguousarray(w_kv[INNER:].T).astype(BF),
        "w_oT": np.ascontiguousarray(w_out.T).astype(BF),
        "w1v": np.ascontiguousarray(np.tile(np.asarray(w_off1, np.float32), G))[:, None],
        "b1v": np.ascontiguousarray(np.tile(np.asarray(b_off1, np.float32), G))[:, None],
        "W2x": np.ascontiguousarray(W2[0]),
        "W2y": np.ascontiguousarray(W2[1]),
        "b_out": np.asarray(b_out, np.float32)[:, None],
    }


def build(stage=5):
    nc = bacc.Bacc("TRN2", target_bir_lowering=False)
    x_in = nc.dram_tensor("x", [DIM, S], BF16, kind="ExternalInput")
    w_qT = nc.dram_tensor("w_qT", [DIM, INNER], BF16, kind="ExternalInput")
    w_kT = nc.dram_tensor("w_kT", [INNER, INNER], BF16, kind="ExternalInput")
    w_vT = nc.dram_tensor("w_vT", [INNER, INNER], BF16, kind="ExternalInput")
    w_oT = nc.dram_tensor("w_oT", [INNER, DIM], BF16, kind="ExternalInput")
    w1v = nc.dram_tensor("w1v", [INNER, 1], F32, kind="ExternalInput")
    b1v = nc.dram_tensor("b1v", [INNER, 1], F32, kind="ExternalInput")
    W2x = nc.dram_tensor("W2x", [INNER, G], F32, kind="ExternalInput")
    W2y = nc.dram_tensor("W2y", [INNER, G], F32, kind="ExternalInput")
    b_out = nc.dram_tensor("b_out", [DIM, 1], F32, kind="ExternalInput")
    y_out = nc.dram_tensor("y", [DIM, S], F32, kind="ExternalOutput")

    meshA_np, meshB_np = host_constants()
    meshA_d = nc.inline_tensor(meshA_np, "meshA")
    meshB_d = nc.inline_tensor(meshB_np, "meshB")

    with tile.TileContext(nc) as tc:
        # ---------------- persistent pool ----------------
        P0 = tc.alloc_tile_pool(name="P0", bufs=1)
        ident = P0.tile([128, 128], F32)
        from concourse.masks import make_identity
        make_identity(nc, ident[:])
        ident_bf = P0.tile([128, 128], BF16)
        nc.vector.tensor_copy(ident_bf[:], ident[:])
        meshA = P0.tile([128, 16], F32); meshB = P0.tile([128, 16], F32)
        nc.sync.dma_start(meshA[:], meshA_d.ap())
        nc.sync.dma_start(meshB[:], meshB_d.ap())
        w1_sb = P0.tile([128, 4], F32); b1_sb = P0.tile([128, 4], F32)
        nc.sync.dma_start(w1_sb[:], w1v.ap().rearrange("(c p) one -> p (c one)", p=128))
        nc.sync.dma_start(b1_sb[:], b1v.ap().rearrange("(c p) one -> p (c one)", p=128))
        W2x_sb = P0.tile([128, 4, G], F32); W2y_sb = P0.tile([128, 4, G], F32)
        nc.sync.dma_start(W2x_sb[:], W2x.ap().rearrange("(c p) g -> p c g", p=128))
        nc.sync.dma_start(W2y_sb[:], W2y.ap().rearrange("(c p) g -> p c g", p=128))
        bout_sb = P0.tile([128, 4], F32)
        nc.sync.dma_start(bout_sb[:], b_out.ap().rearrange("(c p) one -> p (c one)", p=128))
        IDX = P0.tile([128, 32], I32)
        Wb = P0.tile([128, 64], F32)
        kvf = P0.tile([128, 4, J], BF16)
        k_sb = P0.tile([128, 4, J], BF16)
        vT_sb = P0.tile([128, 2, 8, 128], BF16)   # [j, jt, head, (vT|ones)]
        wo_sb = P0.tile([128, 4, DIM], BF16)
        for c in range(4):
            nc.sync.dma_start(wo_sb[:, c, :], w_oT.ap()[c * 128:(c + 1) * 128, :])

        q_pool = tc.alloc_tile_pool(name="qp", bufs=1)
        q_sb = q_pool.tile([128, 4, S], BF16)

        # DRAM scratch
        drp = tc.alloc_tile_pool(name="dr", bufs=1, space="DRAM")
        qt_map = drp.tile([G * MROWS, D], F32)
        recd_p = tc.alloc_tile_pool(name="recd", bufs=3, space="DRAM")
        zt = P0.tile([G, 2, D], F32)
        nc.vector.memset(zt[:], 0.0)
        guard_dst = bass.AP(tensor=qt_map[:].tensor, offset=qt_map[:].offset,
                            ap=[[MROWS * D, G], [(MROWS - 1) * D, 2], [1, D]])
        nc.sync.dma_start(guard_dst, zt[:])

        # ---------------- phase A: load x, qT pass ----------------
        wq_pool = tc.alloc_tile_pool(name="wqp", bufs=1)
        wq_sb = wq_pool.tile([128, 4, INNER], BF16)
        for c in range(4):
            nc.sync.dma_start(wq_sb[:, c, :], w_qT.ap()[c * 128:(c + 1) * 128, :])
        x_pool = tc.alloc_tile_pool(name="xp", bufs=1)
        x_sb = x_pool.tile([128, 4, S], BF16)
        # s-block-major load so qT matmuls can start before the full x arrives
        for q4 in range(4):
            for c in range(4):
                nc.sync.dma_start(
                    x_sb[:, c, q4 * 1024:(q4 + 1) * 1024],
                    x_in.ap()[c * 128:(c + 1) * 128, q4 * 1024:(q4 + 1) * 1024])

        psA = tc.alloc_tile_pool(name="psA", bufs=3, space="PSUM")
        evp = tc.alloc_tile_pool(name="evp", bufs=4)
        for b in range(8):
            s0 = b * 512
            for ch in range(4):  # qT map chunks via matmul (lhsT = x chunk)
                cs = s0 + ch * 128
                pqt = psA.tile([128, INNER], F32, tag="pqt")
                for c in range(4):
                    nc.tensor.matmul(pqt[:], x_sb[:, c, cs:cs + 128], wq_sb[:, c, :],
                                     start=(c == 0), stop=(c == 3))
                ev = evp.tile([128, INNER], F32, tag="qt_ev")
                nc.scalar.activation(ev[:], pqt[:], AF.Copy)
                dst = bass.AP(tensor=qt_map[:].tensor,
                              offset=qt_map[:].offset + (1 + cs) * D,
                              ap=[[D, 128], [MROWS * D, G], [1, D]])
                nc.sync.dma_start(dst, ev[:].rearrange("p (g d) -> p g d", g=G))
        if stage < 2:
            psA.release(); evp.release(); x_pool.release(); wq_pool.release()
            recd_p.release(); drp.release(); q_pool.release(); P0.release()
            nc.compile(); return nc
        # ---------------- offsets ----------------
        offp = tc.alloc_tile_pool(name="offp", bufs=1)
        psOff = tc.alloc_tile_pool(name="psOff", bufs=1, space="PSUM")
        t_sb = offp.tile([128, 4, J], F32)
        for ic in range(4):
            pqd = psOff.tile([128, J], F32, tag="pqd")
            for c in range(4):
                base = x_sb[:, c, :]
                rhs = bass.AP(tensor=base.tensor, offset=base.offset,
                              ap=[list(base.ap[0]), [256, 16], [4, 16]])
                nc.tensor.matmul(pqd[:], wq_sb[:, c, ic * 128:(ic + 1) * 128], rhs,
                                 start=(c == 0), stop=(c == 3))
            nc.scalar.activation(t_sb[:, ic, :], pqd[:], AF.Gelu,
                                 bias=b1_sb[:, ic:ic + 1], scale=w1_sb[:, ic:ic + 1])
        offx = offp.tile([128, 16], F32); offy = offp.tile([128, 16], F32)
        for jt in range(2):
            pxt = psOff.tile([128, G], F32, tag="pxt")
            pyt = psOff.tile([128, G], F32, tag="pyt")
            px = pxt[:]; py = pyt[:]
            for c in range(4):
                nc.tensor.matmul(px, t_sb[:, c, jt * 128:(jt + 1) * 128],
                                 W2x_sb[:, c, :], start=(c == 0), stop=(c == 3))
            for c in range(4):
                nc.tensor.matmul(py, t_sb[:, c, jt * 128:(jt + 1) * 128],
                                 W2y_sb[:, c, :], start=(c == 0), stop=(c == 3))
            nc.scalar.activation(offx[:, jt * 8:(jt + 1) * 8], px, AF.Tanh)
            nc.scalar.activation(offy[:, jt * 8:(jt + 1) * 8], py, AF.Tanh)

        _fc = [0]
        def f16():
            _fc[0] += 1
            return offp.tile([128, 16], F32, name=f"f16_{_fc[0]}", tag=f"f16_{_fc[0]}")

        xs = f16(); ys = f16()
        nc.vector.scalar_tensor_tensor(out=xs[:], in0=offx[:], scalar=4.0 * C15,
                                       in1=meshA[:], op0=OP.mult, op1=OP.add)
        nc.vector.scalar_tensor_tensor(out=ys[:], in0=offy[:], scalar=4.0 * C15,
                                       in1=meshB[:], op0=OP.mult, op1=OP.add)

        def floor_of(src):
            _fc[0] += 1
            ti = offp.tile([128, 16], I32, name=f"i16_{_fc[0]}", tag=f"i16_{_fc[0]}")
            nc.vector.tensor_copy(ti[:], src)
            tf = f16()
            nc.vector.tensor_copy(tf[:], ti[:])
            gt = f16()
            nc.vector.tensor_tensor(out=gt[:], in0=tf[:], in1=src, op=OP.is_gt)
            fl = f16()
            nc.vector.tensor_tensor(out=fl[:], in0=tf[:], in1=gt[:], op=OP.subtract)
            return fl

        x0f = floor_of(xs[:]); y0f = floor_of(ys[:])

        def in_range(v, lo, hi):
            a = f16(); b2 = f16(); r = f16()
            nc.vector.tensor_scalar(out=a[:], in0=v, scalar1=float(lo), scalar2=None,
                                    op0=OP.is_ge)
            nc.vector.tensor_scalar(out=b2[:], in0=v, scalar1=float(hi), scalar2=None,
                                    op0=OP.is_le)
            nc.vector.tensor_tensor(out=r[:], in0=a[:], in1=b2[:], op=OP.mult)
            return r

        vx0 = in_range(x0f[:], 0, 63); vx1 = in_range(x0f[:], -1, 62)
        vy0 = in_range(y0f[:], 0, 63); vy1 = in_range(y0f[:], -1, 62)
        wx1 = f16(); wy1 = f16()
        nc.vector.tensor_tensor(out=wx1[:], in0=xs[:], in1=x0f[:], op=OP.subtract)
        nc.vector.tensor_tensor(out=wy1[:], in0=ys[:], in1=y0f[:], op=OP.subtract)
        wx0m = f16(); wx1m = f16(); wy0m = f16(); wy1m = f16()
        nc.vector.scalar_tensor_tensor(out=wx0m[:], in0=wx1[:], scalar=1.0,
                                       in1=vx0[:], op0=OP.subtract, op1=OP.mult)
        nc.vector.tensor_scalar_mul(wx0m[:], wx0m[:], -1.0)
        nc.vector.tensor_tensor(out=wx1m[:], in0=wx1[:], in1=vx1[:], op=OP.mult)
        nc.vector.scalar_tensor_tensor(out=wy0m[:], in0=wy1[:], scalar=1.0,
                                       in1=vy0[:], op0=OP.subtract, op1=OP.mult)
        nc.vector.tensor_scalar_mul(wy0m[:], wy0m[:], -1.0)
        nc.vector.tensor_tensor(out=wy1m[:], in0=wy1[:], in1=vy1[:], op=OP.mult)
        nc.vector.tensor_tensor(out=Wb[:, 0:16], in0=wy0m[:], in1=wx0m[:], op=OP.mult)
        nc.vector.tensor_tensor(out=Wb[:, 16:32], in0=wy0m[:], in1=wx1m[:], op=OP.mult)
        nc.vector.tensor_tensor(out=Wb[:, 32:48], in0=wy1m[:], in1=wx0m[:], op=OP.mult)
        nc.vector.tensor_tensor(out=Wb[:, 48:64], in0=wy1m[:], in1=wx1m[:], op=OP.mult)
        xm = f16(); ym0 = f16(); ym1 = f16()
        nc.vector.tensor_scalar(out=xm[:], in0=x0f[:], scalar1=-1.0, scalar2=63.0,
                                op0=OP.max, op1=OP.min)
        nc.vector.tensor_scalar(out=ym0[:], in0=y0f[:], scalar1=0.0, scalar2=63.0,
                                op0=OP.max, op1=OP.min)
        nc.vector.tensor_scalar(out=ym1[:], in0=y0f[:], scalar1=1.0, scalar2=0.0,
                                op0=OP.add, op1=OP.max)
        nc.vector.tensor_scalar_min(ym1[:], ym1[:], 63.0)
        IDXf = offp.tile([128, 32], F32)
        nc.vector.scalar_tensor_tensor(out=IDXf[:, 0:16], in0=ym0[:], scalar=64.0,
                                       in1=xm[:], op0=OP.mult, op1=OP.add)
        nc.vector.scalar_tensor_tensor(out=IDXf[:, 16:32], in0=ym1[:], scalar=64.0,
                                       in1=xm[:], op0=OP.mult, op1=OP.add)
        nc.vector.tensor_copy(IDX[:], IDXf[:])

        psOff.release(); psA.release()
        offp.release(); evp.release()

        if stage < 3:
            x_pool.release(); wq_pool.release()
            recd_p.release(); drp.release(); q_pool.release(); P0.release()
            nc.compile(); return nc

        # ---------------- natural q (overlaps the gathers below on PE) -----
        psQ = tc.alloc_tile_pool(name="psQ", bufs=3, space="PSUM")
        for b in range(8):
            s0 = b * 512
            for ic in range(4):
                pq = psQ.tile([128, 512], F32, tag="pq")
                for c in range(4):
                    nc.tensor.matmul(pq[:], wq_sb[:, c, ic * 128:(ic + 1) * 128],
                                     x_sb[:, c, s0:s0 + 512],
                                     start=(c == 0), stop=(c == 3))
                nc.scalar.activation(q_sb[:, ic, s0:s0 + 512], pq[:], AF.Copy)

        # ---------------- gathers + bilinear + kvf ----------------
        gpool = tc.alloc_tile_pool(name="gp", bufs=3)
        psT = tc.alloc_tile_pool(name="psT", bufs=2, space="PSUM")
        qt_flat = qt_map[:]
        for g in range(G):
            Gt = gpool.tile([128, 512], F32, tag="G")
            for yy in range(2):
                for t in range(2):
                    col = yy * 16 + t * 8 + g
                    nc.gpsimd.indirect_dma_start(
                        out=Gt[:, (yy * 2 + t) * 128:(yy * 2 + t + 1) * 128],
                        out_offset=None, in_=qt_flat,
                        in_offset=bass.IndirectOffsetOnAxis(
                            ap=IDX[:, col:col + 1], axis=0),
                        element_offset=(g * MROWS + 1) * D)
            for t in range(2):
                acc = gpool.tile([128, D], BF16, tag="acc")
                m = t * 8 + g
                nc.vector.tensor_scalar(out=acc[:], in0=Gt[:, t * 128:t * 128 + 64],
                                        scalar1=Wb[:, m:m + 1], scalar2=None,
                                        op0=OP.mult)
                for yy, xx in ((0, 1), (1, 0), (1, 1)):
                    blk = (yy * 2 + t) * 128 + xx * 64
                    wcol = (2 * yy + xx) * 16 + m
                    nc.vector.scalar_tensor_tensor(
                        out=acc[:], in0=Gt[:, blk:blk + 64],
                        scalar=Wb[:, wcol:wcol + 1], in1=acc[:],
                        op0=OP.mult, op1=OP.add)
                pt = psT.tile([64, 128], BF16, tag="pt")
                nc.tensor.transpose(pt[:], acc[:], ident_bf[:])
                nc.vector.tensor_copy(
                    kvf[(g % 2) * 64:(g % 2) * 64 + 64, g // 2, t * 128:(t + 1) * 128],
                    pt[:])

        if stage < 4:
            psT.release(); gpool.release(); psQ.release()
            x_pool.release(); wq_pool.release()
            recd_p.release(); drp.release(); q_pool.release(); P0.release()
            nc.compile(); return nc
        # ---------------- k and vT' ----------------
        psT.release(); gpool.release(); psQ.release()
        wkv_pool = tc.alloc_tile_pool(name="wkvp", bufs=1)
        wk_sb = wkv_pool.tile([128, 4, INNER], BF16)
        wv_sb = wkv_pool.tile([128, 4, INNER], BF16)
        for c in range(4):
            nc.sync.dma_start(wk_sb[:, c, :], w_kT.ap()[c * 128:(c + 1) * 128, :])
            nc.sync.dma_start(wv_sb[:, c, :], w_vT.ap()[c * 128:(c + 1) * 128, :])
        psKV = tc.alloc_tile_pool(name="psKV", bufs=2, space="PSUM")
        for oc in range(4):
            pk = psKV.tile([128, J], F32, tag="pk")
            for c in range(4):
                nc.tensor.matmul(pk[:], wk_sb[:, c, oc * 128:(oc + 1) * 128],
                                 kvf[:, c, :], start=(c == 0), stop=(c == 3))
            nc.vector.tensor_copy(k_sb[:, oc, :], pk[:])
        nc.vector.memset(vT_sb[:], 1.0)
        for jt in range(2):
            pv = psKV.tile([128, INNER], F32, tag="pv")
            for c in range(4):
                nc.tensor.matmul(pv[:], kvf[:, c, jt * 128:(jt + 1) * 128],
                                 wv_sb[:, c, :], start=(c == 0), stop=(c == 3))
            for h in range(8):
                # even head -> value cols 0:64 (den rows 64:128);
                # odd head  -> value cols 64:128 (den rows 0:64)
                o0 = (h % 2) * 64
                nc.vector.tensor_copy(vT_sb[:, jt, h, o0:o0 + 64],
                                      pv[:, h * 64:(h + 1) * 64])
        psKV.release(); wkv_pool.release()
        x_pool.release(); wq_pool.release()

        if stage < 5:
            recd_p.release(); drp.release(); q_pool.release(); P0.release()
            nc.compile(); return nc
        # ---------------- attention, pipelined units of (s-half, hp) -------
        outT_pool = tc.alloc_tile_pool(name="otp", bufs=1)
        outT_sb = outT_pool.tile([128, 4, S], BF16)
        ep = tc.alloc_tile_pool(name="ep", bufs=3)
        zrp = tc.alloc_tile_pool(name="zrp", bufs=4)
        pkp = tc.alloc_tile_pool(name="pkp", bufs=3)
        rcp = tc.alloc_tile_pool(name="rcp", bufs=3)
        yev = tc.alloc_tile_pool(name="yev", bufs=3)
        psF = tc.alloc_tile_pool(name="psF", bufs=1, space="PSUM")
        psS = tc.alloc_tile_pool(name="psS", bufs=1, space="PSUM")
        psAV = tc.alloc_tile_pool(name="psAV", bufs=3, space="PSUM")

        NU = 8  # units: u = sh*4 + hp (s-half major so final proj can overlap)
        st = {}  # per-unit tiles

        def issue_E_section(u, sec):
            sh, hp = u // 4, u % 4
            jt, blk = sec // 2, sec % 2
            if sec == 0:
                st[u] = {"E": ep.tile([128, 2, 2, 2048], BF16, tag="E",
                                      name=f"E{u}")}
            E = st[u]["E"]
            ps2 = psS.tile([128, 2, 1024], F32, tag="ps2", name=f"ps2_{u}_{sec}")
            for half in range(2):
                cs = sh * 2048 + blk * 1024 + half * 512
                nc.tensor.matmul(
                    ps2[:, 0, half * 512:(half + 1) * 512],
                    k_sb[0:64, hp, jt * 128:(jt + 1) * 128],
                    q_sb[0:64, hp, cs:cs + 512],
                    start=True, stop=True, tile_position=(0, 0))
                nc.tensor.matmul(
                    ps2[:, 1, half * 512:(half + 1) * 512],
                    k_sb[64:128, hp, jt * 128:(jt + 1) * 128],
                    q_sb[64:128, hp, cs:cs + 512],
                    start=True, stop=True, tile_position=(64, 0))
            eout = bass.AP(
                tensor=E[:].tensor, offset=E[:].offset + jt * 2048 + blk * 1024,
                ap=[list(E[:].ap[0]), [2 * 2048, 2], [1, 1024]])
            nc.scalar.activation(eout, ps2[:], AF.Exp, scale=SCALE)

        def issue_AV_part(u, sec):
            # 2 of the unit's 8 AV tiles per call
            sh, hp = u // 4, u % 4
            E = st[u]["E"]
            if sec == 0:
                st[u]["Z"] = zrp.tile([128, 8, 512], BF16, tag="Z", name=f"Z{u}")
            Z = st[u]["Z"]
            for t in (2 * sec, 2 * sec + 1):
                hh, chunk = t // 4, t % 4
                h = 2 * hp + hh
                pav = psAV.tile([128, 512], F32, tag="pav", name=f"pav{u}_{t}")
                for jt in range(2):
                    nc.tensor.matmul(pav[:], vT_sb[:, jt, h, :],
                                     E[:, hh, jt, chunk * 512:(chunk + 1) * 512],
                                     start=(jt == 0), stop=(jt == 1))
                nc.vector.tensor_copy(Z[:, t, :], pav[:])

        def issue_pack(u):
            # pack den rows (gpsimd cast DMA bf16->f32, off the DVE queue)
            Z = st[u]["Z"]
            pk = pkp.tile([8, 512], F32, tag="pk8", name=f"pk8_{u}")
            # hh0 tiles (t=0..3): den replicated rows 64:128; hh1: rows 0:64
            nc.gpsimd.dma_start(pk[0:4, :], Z[64:65, 0:4, :])
            nc.gpsimd.dma_start(pk[4:8, :], Z[0:1, 4:8, :])
            st[u]["pk"] = pk

        def issue_recbc(u):
            # fast approx reciprocal (f32) + DRAM roundtrip broadcast (bf16)
            rec = pkp.tile([8, 512], F32, tag="rec8", name=f"rec8_{u}")
            nc.vector.reciprocal_approx_fast(rec[:], st[u]["pk"][:])
            rd = recd_p.tile([8, 512], BF16, name=f"rd{u}")
            nc.gpsimd.dma_start(rd[:], rec[:])
            REC = rcp.tile([128, 8, 512], BF16, tag="REC", name=f"REC{u}")
            src = bass.AP(tensor=rd[:].tensor, offset=rd[:].offset,
                          ap=[[0, 128], [512, 8], [1, 512]])
            nc.sync.dma_start(REC[:], src)
            st[u]["REC"] = REC

        def issue_mults(u):
            sh, hp = u // 4, u % 4
            Z = st[u]["Z"]; REC = st[u]["REC"]
            for hh in range(2):
                r0 = hh * 64  # hh0 data rows 0:64, hh1 data rows 64:128
                t0 = hh * 4
                cs = sh * 2048
                nc.vector.tensor_tensor(
                    out=outT_sb[r0:r0 + 64, hp, cs:cs + 2048],
                    in0=Z[r0:r0 + 64, t0:t0 + 4, :], in1=REC[r0:r0 + 64, t0:t0 + 4, :],
                    op=OP.mult)

        def issue_final_group(sh, grp):
            # one (oc, sb2) group of the final projection for s-half sh
            oc, sbi = grp // 4, grp % 4
            sb2 = sh * 4 + sbi
            pf = psF.tile([128, 512], F32, tag="pf", name=f"pf{sh}_{grp}")
            for ic in range(4):
                nc.tensor.matmul(pf[:], wo_sb[:, ic, oc * 128:(oc + 1) * 128],
                                 outT_sb[:, ic, sb2 * 512:(sb2 + 1) * 512],
                                 start=(ic == 0), stop=(ic == 3))
            ye = yev.tile([128, 512], F32, tag="ye")
            nc.vector.tensor_scalar(out=ye[:], in0=pf[:],
                                    scalar1=bout_sb[:, oc:oc + 1], scalar2=None,
                                    op0=OP.add)
            nc.scalar.dma_start(
                y_out.ap()[oc * 128:(oc + 1) * 128, sb2 * 512:(sb2 + 1) * 512],
                ye[:])

        # pipeline: E(u) | AV(u-1)+Zevac | pack(u-1) | recip/REC(u-2) | mults(u-3)
        # final proj for s-half 0 (ready after mults of unit 3, i.e. u>=7)
        # interleaves 2 groups per sec during u=7..8; s-half 1 runs at the end.
        for u in range(NU + 2):
            for sec in range(4):
                if u < NU:
                    issue_E_section(u, sec)
                if 0 <= u - 1 < NU:
                    issue_AV_part(u - 1, sec)
                if u in (7, 8):
                    g0 = (u - 7) * 8 + sec * 2
                    issue_final_group(0, g0)
                    issue_final_group(0, g0 + 1)
            if 0 <= u - 1 < NU:
                issue_pack(u - 1)
            if 0 <= u - 2 < NU:
                issue_recbc(u - 2)
            if 0 <= u - 3 < NU:
                issue_mults(u - 3)
        # drain: unit 7's norm chain, compressed
        issue_recbc(NU - 1)
        issue_mults(NU - 2)
        issue_mults(NU - 1)

        psAV.release(); psS.release()
        psF2 = tc.alloc_tile_pool(name="psF2", bufs=4, space="PSUM")

        def issue_final_group2(sh, grp):
            oc, sbi = grp // 4, grp % 4
            sb2 = sh * 4 + sbi
            pf = psF2.tile([128, 512], F32, tag="pf2", name=f"pf2_{sh}_{grp}")
            for ic in range(4):
                nc.tensor.matmul(pf[:], wo_sb[:, ic, oc * 128:(oc + 1) * 128],
                                 outT_sb[:, ic, sb2 * 512:(sb2 + 1) * 512],
                                 start=(ic == 0), stop=(ic == 3))
            ye = yev.tile([128, 512], F32, tag="ye")
            nc.vector.tensor_scalar(out=ye[:], in0=pf[:],
                                    scalar1=bout_sb[:, oc:oc + 1], scalar2=None,
                                    op0=OP.add)
            nc.scalar.dma_start(
                y_out.ap()[oc * 128:(oc + 1) * 128, sb2 * 512:(sb2 + 1) * 512],
                ye[:])

        for grp in range(16):
            issue_final_group2(1, grp)

        psF2.release(); psF.release()
        yev.release(); rcp.release(); pkp.release()
        zrp.release(); ep.release()
        outT_pool.release()
        recd_p.release(); drp.release(); q_pool.release(); P0.release()
    nc.compile()
    return nc


# ---------------------------------------------------------------------------
# Public entry point: full (unsharded) inputs -> full output.
# Data-parallel over batch: image i runs on NeuronCore i (8 cores).
# ---------------------------------------------------------------------------
_NC_CACHE = {}


def _get_nc():
    if "nc" not in _NC_CACHE:
        _NC_CACHE["nc"] = build()
    return _NC_CACHE["nc"]


def kernel(x, w_q, w_off1, b_off1, w_off2, w_kv, w_out, b_out):
    from concourse.bass_utils import run_bass_kernel_spmd
    x = np.asarray(x, np.float32)
    b = x.shape[0]
    assert x.shape == (8, DIM, H, W), f"unexpected x shape {x.shape}"
    wd = prep_weights(w_q, w_off1, b_off1, w_off2, w_kv, w_out, b_out)
    in_maps = [{"x": np.ascontiguousarray(x[i].reshape(DIM, S)).astype(BF), **wd}
               for i in range(b)]
    nc = _get_nc()
    res = run_bass_kernel_spmd(nc, in_maps, core_ids=list(range(b)))
    out = np.stack([res.results[i]["y"].reshape(DIM, H, W) for i in range(b)])
    return out.astype(np.float32)
